# revision 38
# baseline (speedup 1.0000x reference)
"""Trainium2 Bass kernel for nn_EnhancedLocalAttention.

Reference semantics (B=4, L=4096, C=1024, H=16, D=64, WIN=256, step=128):
  qkv = x @ W_qkv + b_qkv -> q,k,v [B,H,L,D]
  overlapping windows n: tokens [n*128, n*128+256)
  per (b,h,n): S = (Q_win^T K_win)/8  (D x D, contracted over the 256 window
  tokens), P = softmax(S, axis=-1), O = P @ V_win^T  (D x W)
  regroup: rows of reshape(O, [256, 64]) laid at tokens n*256..n*256+255,
  slice to L -> only windows 0..15 survive; then @ W_out + b_out.

Sharding: 8 cores = (4 batches) x (2 window-halves of 8 windows each).
Each core consumes 9 x 128-token chunks and produces 2048 output rows.

v17 (over v6 baseline; 241us -> ~204us):
  - host pre-transposes x into SBUF-layout chunks (no xbar transposes) and
    pre-packs weights per projection (wq | wk | wv | wo) in consumption order
  - PE clock warmup: ~11 dummy matmuls burn the tensor engine's p-state
    ramp while the first input DMAs are in flight
  - weight DMAs issued at the head of the Sync (hw-DGE) stream, x chunks on
    Scalar's: transfers start at preamble-end in PE consumption order, and
    DGE-ring backpressure parks Sync (idle anyway) instead of Scalar, which
    must run psum evictions + EXPs from ~13us
  - QKV projection split into separate Q / K / V psum passes (2 banks each);
    within a pass each psum bank's accumulation group stays sequential (the
    PE cannot interleave accumulation groups within one bank)
  - prologue covers chunks 0-3 Q/K + windows 0-1 S-phase + chunks 0-1 V,
    ordered by DMA arrival (all Q passes before any K pass) so the PE is
    never parked behind a matmul whose weights haven't landed
  - steady state: round r runs QKV(r) + S-phase of window r-2 + P^T/O/
    out-proj of window r-3; softmax chains run a full round before use;
    block-interleave (8 qkv units : 4 window units) keeps transitions
    between small and large matmuls (exposed LDWEIGHTS) down
  - tail: sphase(7) rides round 8; the final round runs phase2(6)/(7)
    interleaved so their copy chains hide behind each other's matmuls
"""

import threading

import numpy as np

import concourse.bacc as bacc
import concourse.masks as masks
import concourse.mybir as mybir
import concourse.tile as tile
from concourse._compat import get_trn_type
from concourse.bass_utils import run_bass_kernel_spmd

F32 = mybir.dt.float32
F16 = mybir.dt.float16
EXP = mybir.ActivationFunctionType.Exp
AXX = mybir.AxisListType.X

B, L, C = 4, 4096, 1024
H, D, WIN, STEP = 16, 64, 256, 128
NCHUNK = 9            # 128-token chunks per core
NWIN = 8              # windows per core
TOK = NCHUNK * 128    # 1152 input tokens per core
OUT_ROWS = NWIN * 256 # 2048 output rows per core


def interleave(a, b):
    """Merge two unit lists proportionally (Bresenham)."""
    if not b:
        return list(a)
    if not a:
        return list(b)
    out = []
    ia = ib = 0
    while ia < len(a) or ib < len(b):
        if ib >= len(b) or (ia < len(a) and ia * len(b) <= ib * len(a)):
            out.append(a[ia]); ia += 1
        else:
            out.append(b[ib]); ib += 1
    return out


def interleave_blk(a, b, ka, kb):
    """Block-wise Bresenham merge: fewer small<->large matmul transitions
    (each transition exposes the next stationary-weight load)."""
    ab = [a[i : i + ka] for i in range(0, len(a), ka)]
    bb = [b[i : i + kb] for i in range(0, len(b), kb)]
    return [u for blk in interleave(ab, bb) for u in blk]


def build_program(with_bias=False):
    nc = bacc.Bacc(
        get_trn_type() or "TRN2",
        target_bir_lowering=False,
        debug=False,
        num_devices=8,
    )
    # xs is pre-transposed on host: row r*128+p, col cb*128+t holds
    # x[r*128+t, cb*128+p], so each chunk tile loads with a plain DMA.
    xs = nc.dram_tensor("xs", [TOK, C], F16, kind="ExternalInput")
    wq = nc.dram_tensor("wq", [C, C], F16, kind="ExternalInput")
    wk = nc.dram_tensor("wk", [C, C], F16, kind="ExternalInput")
    wv = nc.dram_tensor("wv", [C, C], F16, kind="ExternalInput")
    wo = nc.dram_tensor("wo", [C, C], F16, kind="ExternalInput")
    bqkv = nc.dram_tensor("bqkv", [3 * C], F32, kind="ExternalInput")
    bout = nc.dram_tensor("bout", [C], F32, kind="ExternalInput")
    out = nc.dram_tensor("out", [OUT_ROWS, C], F32, kind="ExternalOutput")

    from contextlib import ExitStack

    with tile.TileContext(nc) as tc, ExitStack() as ctx:
        pool = lambda name, bufs: ctx.enter_context(tc.tile_pool(name=name, bufs=bufs))
        wq_pool = pool("wq", 8)
        wk_pool = pool("wk", 4)
        wv_pool = pool("wv", 4)
        wo_pool = pool("wo", 2)
        const_pool = pool("const", 1)
        xt_pool = pool("xt", 5)
        q_pool = pool("q", 5)
        k_pool = pool("k", 5)
        vt_pool = pool("vt", 6)
        at_pool = pool("at", 8)
        st_pool = pool("st", 8)
        yt_pool = pool("yt", 8)
        o_pool = pool("o", 3)
        ps_a = ctx.enter_context(tc.tile_pool(name="psa", bufs=4, space="PSUM"))
        ps_b = ctx.enter_context(tc.tile_pool(name="psb", bufs=4, space="PSUM"))

        # --- PE clock warmup: the tensor engine starts at a low p-state and
        # only reaches full clock after ~3us of continuous work. Burn that
        # ramp on dummy matmuls while the first input DMAs are in flight, so
        # the real prologue runs at full speed. ---
        warm = const_pool.tile([128, 512], F16, tag="warm", name="warm")
        nc.vector.memset(warm[:], 0.0)
        wpsum = ps_a.tile([128, 512], F32, tag="a", name="wpsum")
        for _ in range(11):
            nc.tensor.matmul(
                wpsum[:], warm[:, 0:128], warm[:], start=True, stop=True
            )

        # --- input DMAs first: x chunks on Scalar, weights on Sync (both
        # hardware DGE queues), in PE consumption order. Weights go on Sync
        # because DGE-ring backpressure parks the issuing engine for ~20us;
        # Sync has no early compute, while Scalar must run the Q-pass psum
        # evictions and EXPs from ~13us on. ---
        xt_all = [None] * NCHUNK

        def prefetch_xt(r):
            xtt = xt_pool.tile([128, C], F16, tag="xt", name="xtt")
            nc.scalar.dma_start(xtt[:], xs.ap()[r * 128 : (r + 1) * 128, :])
            xt_all[r] = xtt

        for r in range(5):
            prefetch_xt(r)

        def load_w2(pool_, dram, j, nm):
            # [256, 1024] DRAM rows -> one [128, 2048] tile (2 cb blocks)
            t = pool_.tile([128, 2 * C], F16, tag=nm, name=f"{nm}{j}")
            nc.sync.dma_start(
                t[:].rearrange("p (two f) -> p two f", two=2),
                dram.ap()[j * 256 : (j + 1) * 256, :].rearrange(
                    "(two p) f -> p two f", two=2
                ),
            )
            return t

        def load_w4(pool_, dram, g, nm):
            # [512, 1024] DRAM rows -> one [128, 4096] tile (4 cb blocks)
            t = pool_.tile([128, 4 * C], F16, tag=nm, name=f"{nm}{g}")
            nc.sync.dma_start(
                t[:].rearrange("p (four f) -> p four f", four=4),
                dram.ap()[g * 512 : (g + 1) * 512, :].rearrange(
                    "(four p) f -> p four f", four=4
                ),
            )
            return t

        def load_w1(pool_, dram, cb, nm):
            # single-cb granule: smallest arrival quantum for the first blocks
            t = pool_.tile([128, C], F16, tag=nm, name=f"{nm}{cb}")
            nc.sync.dma_start(t[:], dram.ap()[cb * 128 : (cb + 1) * 128, :])
            return t

        wq_sb = [load_w1(wq_pool, wq, cb, "wq") for cb in range(8)]
        wk_sb = [load_w2(wk_pool, wk, j, "wk") for j in range(4)]
        wv_sb = [load_w2(wv_pool, wv, j, "wv") for j in range(4)]
        wo_sb = [load_w4(wo_pool, wo, g, "wo") for g in range(2)]

        def wq_blk(cb, i):   # Q-proj rhs: cols i*512..+512 of cb's Q block
            return wq_sb[cb][:, i * 512 : (i + 1) * 512]

        def wk_blk(cb, i):
            return wk_sb[cb // 2][:, (cb % 2) * C + i * 512 : (cb % 2) * C + (i + 1) * 512]

        def wv_blk(cb, hp):  # V-proj lhsT: head-pair hp cols of cb's block
            base = (cb % 2) * C + hp * 128
            return wv_sb[cb // 2][:, base : base + 128]

        def wo_blk(cb, mi):
            base = (cb % 4) * C + mi * 512
            return wo_sb[cb // 4][:, base : base + 512]

        # --- constants (vector/gpsimd streams; after DMA issues) ---
        idf16 = const_pool.tile([128, 128], F16, tag="idf16", name="idf16")
        masks.make_identity(nc, idf16[:])
        ones = const_pool.tile([1, 128], F16, tag="ones", name="ones")
        nc.vector.memset(ones[:], 1.0)
        bq_sb = const_pool.tile([1, 3 * C], F16, tag="bq", name="bq_sb")
        bo_sb = const_pool.tile([1, C], F16, tag="bo", name="bo_sb")
        if with_bias:
            nc.gpsimd.dma_start(bq_sb[:], bqkv.ap().rearrange("(a f) -> a f", a=1))
            nc.gpsimd.dma_start(bo_sb[:], bout.ap().rearrange("(a f) -> a f", a=1))

        def xt_blk(r, cb):
            return xt_all[r][:, cb * 128 : (cb + 1) * 128]

        q_sb = [None] * NCHUNK
        k_sb = [None] * NCHUNK
        vt_sb = [None] * NCHUNK   # [e-pair 128, hp*128 + tok]
        wstate = [
            {"pe4": [None, None], "rs4": [None, None]} for _ in range(NWIN)
        ]

        def q_units(r):
            """Chunk r Q projection: 2 psum banks, cb-progressive."""
            st = {}

            def u_alloc():
                st["p"] = [
                    ps_a.tile([128, 512], F32, tag="a", name=f"pq{i}")
                    for i in range(2)
                ]

            def u_cb(cb):
                def f():
                    for i in range(2):
                        nc.tensor.matmul(
                            st["p"][i][:],
                            xt_blk(r, cb),
                            wq_blk(cb, i),
                            start=(cb == 0),
                            stop=(not with_bias and cb == 7),
                        )
                return f

            def u_fin():
                if with_bias:
                    for i in range(2):
                        nc.tensor.matmul(
                            st["p"][i][:],
                            ones[:, :],
                            bq_sb[:, i * 512 : (i + 1) * 512],
                            start=False,
                            stop=True,
                        )
                qt = q_pool.tile([128, C], F16, tag="q", name="qt")
                nc.scalar.mul(qt[:, 0:512], st["p"][0][:], 0.125)
                nc.scalar.mul(qt[:, 512:1024], st["p"][1][:], 0.125)
                q_sb[r] = qt

            return [u_alloc] + [u_cb(cb) for cb in range(8)] + [u_fin]

        def k_units(r):
            st = {}

            def u_alloc():
                st["p"] = [
                    ps_a.tile([128, 512], F32, tag="a", name=f"pk{i}")
                    for i in range(2)
                ]

            def u_cb(cb):
                def f():
                    for i in range(2):
                        nc.tensor.matmul(
                            st["p"][i][:],
                            xt_blk(r, cb),
                            wk_blk(cb, i),
                            start=(cb == 0),
                            stop=(not with_bias and cb == 7),
                        )
                return f

            def u_fin():
                if with_bias:
                    for i in range(2):
                        nc.tensor.matmul(
                            st["p"][i][:],
                            ones[:, :],
                            bq_sb[:, C + i * 512 : C + (i + 1) * 512],
                            start=False,
                            stop=True,
                        )
                kt = k_pool.tile([128, C], F16, tag="k", name="kt")
                nc.vector.tensor_copy(kt[:, 0:512], st["p"][0][:])
                nc.vector.tensor_copy(kt[:, 512:1024], st["p"][1][:])
                k_sb[r] = kt

            return [u_alloc] + [u_cb(cb) for cb in range(8)] + [u_fin]

        def v_units(r, ps=None, tag="b"):
            """Chunk r V^T projection. hp-outer: each psum slot's
            accumulation group (cb 0..7) runs without another group
            interleaving on the same bank — PSUM accumulation groups must
            be sequential per bank."""
            st = {}
            ps_pool = ps if ps is not None else ps_b

            def u_alloc():
                st["p"] = [
                    ps_pool.tile([128, 512], F32, tag=tag, name=f"pv{i}")
                    for i in range(2)
                ]

            def u_hp(hp):
                def f():
                    pv = st["p"][hp // 4]
                    sl = (hp % 4) * 128
                    for cb in range(8):
                        nc.tensor.matmul(
                            pv[:, sl : sl + 128],
                            wv_blk(cb, hp),
                            xt_blk(r, cb),
                            start=(cb == 0),
                            stop=(not with_bias and cb == 7),
                        )
                    if with_bias:
                        nc.tensor.matmul(
                            pv[:, sl : sl + 128],
                            bq_sb[:, 2 * C + hp * 128 : 2 * C + (hp + 1) * 128],
                            ones[:, :],
                            start=False,
                            stop=True,
                        )
                return f

            def u_fin():
                v_t = vt_pool.tile([128, C], F16, tag="vt", name="v_t")
                nc.vector.tensor_copy(v_t[:, 0:512], st["p"][0][:])
                nc.scalar.copy(v_t[:, 512:1024], st["p"][1][:])
                vt_sb[r] = v_t

            return [u_alloc] + [u_hp(hp) for hp in range(8)] + [u_fin]

        def qkv_units(r):
            units = []
            if r + 2 < NCHUNK and r + 2 >= 5:
                units.append(lambda: prefetch_xt(r + 2))
            units += q_units(r) + k_units(r) + v_units(r)
            return units

        def sphase_units(w):
            """Window w scores: S matmuls (4 head-pairs per PSUM bank),
            batched EXP / rowsum / reciprocal. Results land in SBUF for
            next round's phase2."""
            ws = wstate[w]

            def u_sb(j):
                def f():
                    sbt = ps_b.tile([128, 512], F32, tag="b", name="sbt")
                    for ii in range(4):
                        hp = 4 * j + ii
                        s = sbt[:, ii * 128 : (ii + 1) * 128]
                        for rr, (b0, b1) in (
                            (w, (True, False)),
                            (w + 1, (False, True)),
                        ):
                            nc.tensor.matmul(
                                s,
                                q_sb[rr][:, hp * 128 : (hp + 1) * 128],
                                k_sb[rr][:, hp * 128 : (hp + 1) * 128],
                                start=b0,
                                stop=b1,
                            )
                    pe4 = at_pool.tile([128, 256], F16, tag="pe4", name="pe4")
                    sb3 = sbt[:].rearrange("p (h c) -> p h c", h=4)
                    pe3 = pe4[:].rearrange("p (h e) -> p h e", h=4)
                    nc.scalar.activation(pe3[0:64], sb3[0:64, :, 0:64], EXP)
                    nc.scalar.activation(pe3[64:128], sb3[64:128, :, 64:128], EXP)
                    ssum4 = st_pool.tile([128, 4], F32, tag="ssum", name="ssum4")
                    nc.vector.reduce_sum(ssum4[:], pe3, axis=AXX)
                    rs4 = st_pool.tile([128, 4], F32, tag="rs", name="rs4")
                    nc.vector.reciprocal(rs4[:], ssum4[:])
                    ws["pe4"][j] = pe4
                    ws["rs4"][j] = rs4
                return f

            return [u_sb(0), u_sb(1)]

        def phase2_units(w):
            """Window w: normalize, P^T, O, out-projection (chain results
            from last round's S-phase)."""
            ws = wstate[w]
            yt2 = [None] * 4
            ptsb = [None] * 4
            ptw = [None] * 2
            yw = [None] * 4

            def u_pt(pp):
                def f():
                    if pp % 2 == 0:
                        ptw[pp // 2] = ps_b.tile(
                            [128, 512], F32, tag="b", name="ptw"
                        )
                    ptp2 = ptw[pp // 2][:, (pp % 2) * 128 : (pp % 2) * 128 + 128]
                    j, pe4, rs4 = pp // 2, ws["pe4"][pp // 2], ws["rs4"][pp // 2]
                    for i in (0, 1):
                        hp = 2 * pp + i
                        ii = hp % 4
                        p_n = at_pool.tile([128, 64], F16, tag="p_n", name="p_n")
                        nc.vector.tensor_scalar_mul(
                            p_n[:],
                            pe4[:, ii * 64 : (ii + 1) * 64],
                            rs4[:, ii : ii + 1],
                        )
                        nc.tensor.matmul(
                            ptp2[0:64, i * 64 : (i + 1) * 64],
                            p_n[0:64, :],
                            idf16[0:64, 0:64],
                            start=True,
                            stop=True,
                            tile_position=(0, 0),
                        )
                        nc.tensor.matmul(
                            ptp2[64:128, i * 64 : (i + 1) * 64],
                            p_n[64:128, :],
                            idf16[64:128, 64:128],
                            start=True,
                            stop=True,
                            tile_position=(64, 64),
                        )
                    pt2 = at_pool.tile([128, 128], F16, tag="ptsb", name="pt2")
                    eng = nc.vector.tensor_copy if pp % 2 else nc.scalar.copy
                    eng(pt2[:], ptp2)
                    ptsb[pp] = pt2
                return f

            def u_o(hp):
                def f():
                    if hp % 2 == 0:
                        yw[hp // 2] = ps_b.tile(
                            [128, 512], F32, tag="b", name="yw"
                        )
                    ypsum = yw[hp // 2][:, (hp % 2) * 256 : (hp % 2) * 256 + 256]
                    pt2 = ptsb[hp // 2]
                    c0 = (hp % 2) * 64
                    for po in (0, 64):
                        rh = pt2[po : po + 64, c0 : c0 + 64]
                        for wq_ in range(4):
                            vtt = vt_sb[w + wq_ // 2]
                            col = hp * 128 + (wq_ % 2) * 64
                            nc.tensor.matmul(
                                ypsum[po : po + 64, wq_ * 64 : (wq_ + 1) * 64],
                                vtt[po : po + 64, col : col + 64],
                                rh,
                                start=True,
                                stop=True,
                                tile_position=(po, po),
                            )
                    if hp % 2 == 1:
                        # Y^T[c, g*256 + d*4+wq] = yw[c, g*256 + wq*64+d]
                        ytt = yt_pool.tile([128, 512], F16, tag="yt", name="ytt")
                        eng = (
                            nc.vector.tensor_copy
                            if (hp // 2) % 2
                            else nc.scalar.copy
                        )
                        eng(
                            ytt[:].rearrange("p (g b a) -> p g a b", g=2, a=4),
                            yw[hp // 2][:].rearrange(
                                "p (g a b) -> p g a b", g=2, a=4
                            ),
                        )
                        yt2[hp // 2] = ytt
                return f

            def u_op(th):
                def f():
                    po_m = [
                        ps_a.tile([128, 512], F32, tag="a", name=f"pom{i}")
                        for i in range(2)
                    ]
                    for cb in range(8):
                        lh = yt2[cb // 2][
                            :, (cb % 2) * 256 + th * 128 : (cb % 2) * 256 + th * 128 + 128
                        ]
                        for mi in range(2):
                            nc.tensor.matmul(
                                po_m[mi][:],
                                lh,
                                wo_blk(cb, mi),
                                start=(cb == 0),
                                stop=(not with_bias and cb == 7),
                            )
                    if with_bias:
                        for mi in range(2):
                            nc.tensor.matmul(
                                po_m[mi][:],
                                ones[:, :],
                                bo_sb[:, mi * 512 : (mi + 1) * 512],
                                start=False,
                                stop=True,
                            )
                    ot = o_pool.tile([128, C], F32, tag="o", name="ot")
                    nc.vector.tensor_copy(ot[:, 0:512], po_m[0][:])
                    nc.scalar.copy(ot[:, 512:1024], po_m[1][:])
                    row = w * 256 + th * 128
                    nc.sync.dma_start(out.ap()[row : row + 128, :], ot[:])
                return f

            return [
                u_pt(0), u_pt(1), u_o(0), u_o(1), u_o(2), u_o(3),
                u_pt(2), u_pt(3), u_o(4), u_o(5), u_o(6), u_o(7),
                u_op(0), u_op(1),
            ]

        # --- prologue: chunks 0-2, interleaved pairwise per projection pass
        # so the PE consumes weight blocks as they arrive (wq -> wk -> wv
        # arrival order); V2 is deferred to round 3 (first use: round 4) ---
        def emit_pair(u0, u1):
            units = [u0[0], u1[0]]                   # allocs (2+2 psum banks)
            for i in range(8):
                units += [u0[1 + i], u1[1 + i]]
            units += [u0[9], u1[9]]                  # fins (evict -> banks free)
            return units

        # All Q passes before any K pass: Q only needs wq (first in the DMA
        # queue), so the PE never sits behind a K matmul waiting for the wk
        # tail. sphase(0)/(1) run before the V allocs (their sbt psum banks
        # are claimed by V0/V1, so the EXP readers must be emitted first).
        for u in (
            emit_pair(q_units(0), q_units(1))
            + emit_pair(q_units(2), q_units(3))
            + emit_pair(k_units(0), k_units(1))
            + sphase_units(0)
            + emit_pair(k_units(2), k_units(3))
            + sphase_units(1)
            + emit_pair(v_units(0), v_units(1))
        ):
            u()

        # --- steady state. Round 3 projects only V2/V3 (Q/K of chunks 2-3
        # ran in the prologue; V3 uses ps_a, which round 3's Q/K no longer
        # occupy, keeping psum reuse gated by emitted evictions). Rounds
        # 4..8 run full QKV. sphase(7) rides round 8's tail; the final
        # round interleaves phase2(6)/(7) with a head start for (6) ---
        for u in v_units(2):
            u()
        r3 = [lambda: prefetch_xt(5)] + v_units(3, ps=ps_a, tag="a")
        for u in interleave_blk(r3, phase2_units(0), 6, 3):
            u()
        ph6 = phase2_units(NWIN - 2)
        for r in range(4, NCHUNK):
            win = sphase_units(r - 2) + phase2_units(r - 3)
            if r == NCHUNK - 1:
                # window 7's S-phase and the first half of window 6's
                # phase2 ride round 8's tail, shrinking the final round
                win += sphase_units(NWIN - 1) + ph6[:6]
            for u in interleave_blk(qkv_units(r), win, 6, 3):
                u()
        ph7 = phase2_units(NWIN - 1)
        for u in interleave(ph6[6:], ph7):
            u()

    nc.compile()
    return nc


_CACHE = {}
_LOCK = threading.Lock()


def _get_program(with_bias=False):
    key = f"nc_bias{with_bias}"
    with _LOCK:
        if key not in _CACHE:
            _CACHE[key] = build_program(with_bias=with_bias)
        return _CACHE[key]


def make_in_maps(x, W_qkv, b_qkv, W_out, b_out):
    x16 = np.asarray(x, dtype=np.float16)
    wqkv16 = np.asarray(W_qkv, dtype=np.float16)
    wout16 = np.ascontiguousarray(np.asarray(W_out, dtype=np.float16))
    bqkv = np.asarray(b_qkv, dtype=np.float32)
    bout = np.asarray(b_out, dtype=np.float32)
    wq16 = np.ascontiguousarray(wqkv16[:, 0:C])
    wk16 = np.ascontiguousarray(wqkv16[:, C : 2 * C])
    wv16 = np.ascontiguousarray(wqkv16[:, 2 * C : 3 * C])
    in_maps = []
    for cid in range(8):
        b, half = cid // 2, cid % 2
        t0 = half * NWIN * STEP
        # xs_prep[r*128+p, cb*128+t] = x[b, t0 + r*128 + t, cb*128 + p]
        xT = x16[b, t0 : t0 + TOK, :].T                      # [C, TOK]
        xs_prep = np.ascontiguousarray(
            xT.reshape(8, 128, NCHUNK, 128).transpose(2, 1, 0, 3).reshape(TOK, C)
        )
        in_maps.append(
            {
                "xs": xs_prep,
                "wq": wq16,
                "wk": wk16,
                "wv": wv16,
                "wo": wout16,
                "bqkv": bqkv,
                "bout": bout,
            }
        )
    return in_maps


def kernel(x, W_qkv, b_qkv, W_out, b_out):
    with_bias = bool(np.any(b_qkv)) or bool(np.any(b_out))
    nc = _get_program(with_bias=with_bias)
    in_maps = make_in_maps(x, W_qkv, b_qkv, W_out, b_out)
    res = run_bass_kernel_spmd(nc, in_maps, core_ids=list(range(8)))
    out_full = np.empty((B, L, C), dtype=np.float32)
    for cid in range(8):
        b, half = cid // 2, cid % 2
        out_full[b, half * OUT_ROWS : (half + 1) * OUT_ROWS, :] = res.results[cid][
            "out"
        ]
    return out_full


# revision 40
# speedup vs baseline: 1.0066x; 1.0066x over previous
"""Trainium2 Bass kernel for nn_EnhancedLocalAttention.

Reference semantics (B=4, L=4096, C=1024, H=16, D=64, WIN=256, step=128):
  qkv = x @ W_qkv + b_qkv -> q,k,v [B,H,L,D]
  overlapping windows n: tokens [n*128, n*128+256)
  per (b,h,n): S = (Q_win^T K_win)/8  (D x D, contracted over the 256 window
  tokens), P = softmax(S, axis=-1), O = P @ V_win^T  (D x W)
  regroup: rows of reshape(O, [256, 64]) laid at tokens n*256..n*256+255,
  slice to L -> only windows 0..15 survive; then @ W_out + b_out.

Sharding: 8 cores = (4 batches) x (2 window-halves of 8 windows each).
Each core consumes 9 x 128-token chunks and produces 2048 output rows.

v17 (over v6 baseline; 241us -> ~204us):
  - host pre-transposes x into SBUF-layout chunks (no xbar transposes) and
    pre-packs weights per projection (wq | wk | wv | wo) in consumption order
  - PE clock warmup: ~11 dummy matmuls burn the tensor engine's p-state
    ramp while the first input DMAs are in flight
  - weight DMAs issued at the head of the Sync (hw-DGE) stream, x chunks on
    Scalar's: transfers start at preamble-end in PE consumption order, and
    DGE-ring backpressure parks Sync (idle anyway) instead of Scalar, which
    must run psum evictions + EXPs from ~13us
  - QKV projection split into separate Q / K / V psum passes (2 banks each);
    within a pass each psum bank's accumulation group stays sequential (the
    PE cannot interleave accumulation groups within one bank)
  - prologue covers chunks 0-3 Q/K + windows 0-1 S-phase + chunks 0-1 V,
    ordered by DMA arrival (all Q passes before any K pass) so the PE is
    never parked behind a matmul whose weights haven't landed
  - steady state: round r runs QKV(r) + S-phase of window r-2 + P^T/O/
    out-proj of window r-3; softmax chains run a full round before use;
    block-interleave (8 qkv units : 4 window units) keeps transitions
    between small and large matmuls (exposed LDWEIGHTS) down
  - tail: sphase(7) rides round 8; the final round runs phase2(6)/(7)
    interleaved so their copy chains hide behind each other's matmuls
"""

import threading

import numpy as np

import concourse.bacc as bacc
import concourse.masks as masks
import concourse.mybir as mybir
import concourse.tile as tile
from concourse._compat import get_trn_type
from concourse.bass_utils import run_bass_kernel_spmd

F32 = mybir.dt.float32
F16 = mybir.dt.float16
EXP = mybir.ActivationFunctionType.Exp
AXX = mybir.AxisListType.X

B, L, C = 4, 4096, 1024
H, D, WIN, STEP = 16, 64, 256, 128
NCHUNK = 9            # 128-token chunks per core
NWIN = 8              # windows per core
TOK = NCHUNK * 128    # 1152 input tokens per core
OUT_ROWS = NWIN * 256 # 2048 output rows per core


def interleave(a, b):
    """Merge two unit lists proportionally (Bresenham)."""
    if not b:
        return list(a)
    if not a:
        return list(b)
    out = []
    ia = ib = 0
    while ia < len(a) or ib < len(b):
        if ib >= len(b) or (ia < len(a) and ia * len(b) <= ib * len(a)):
            out.append(a[ia]); ia += 1
        else:
            out.append(b[ib]); ib += 1
    return out


def interleave_blk(a, b, ka, kb):
    """Block-wise Bresenham merge: fewer small<->large matmul transitions
    (each transition exposes the next stationary-weight load)."""
    ab = [a[i : i + ka] for i in range(0, len(a), ka)]
    bb = [b[i : i + kb] for i in range(0, len(b), kb)]
    return [u for blk in interleave(ab, bb) for u in blk]


def build_program(with_bias=False):
    nc = bacc.Bacc(
        get_trn_type() or "TRN2",
        target_bir_lowering=False,
        debug=False,
        num_devices=8,
    )
    # xs is pre-transposed on host: row r*128+p, col cb*128+t holds
    # x[r*128+t, cb*128+p], so each chunk tile loads with a plain DMA.
    xs = nc.dram_tensor("xs", [TOK, C], F16, kind="ExternalInput")
    wq = nc.dram_tensor("wq", [C, C], F16, kind="ExternalInput")
    wk = nc.dram_tensor("wk", [C, C], F16, kind="ExternalInput")
    wv = nc.dram_tensor("wv", [C, C], F16, kind="ExternalInput")
    wo = nc.dram_tensor("wo", [C, C], F16, kind="ExternalInput")
    bqkv = nc.dram_tensor("bqkv", [3 * C], F32, kind="ExternalInput")
    bout = nc.dram_tensor("bout", [C], F32, kind="ExternalInput")
    out = nc.dram_tensor("out", [OUT_ROWS, C], F32, kind="ExternalOutput")

    from contextlib import ExitStack

    with tile.TileContext(nc) as tc, ExitStack() as ctx:
        pool = lambda name, bufs: ctx.enter_context(tc.tile_pool(name=name, bufs=bufs))
        wq_pool = pool("wq", 8)
        wk_pool = pool("wk", 4)
        wv_pool = pool("wv", 4)
        wo_pool = pool("wo", 2)
        const_pool = pool("const", 1)
        xt_pool = pool("xt", 5)
        q_pool = pool("q", 5)
        k_pool = pool("k", 5)
        vt_pool = pool("vt", 6)
        at_pool = pool("at", 8)
        pn_pool = pool("pn", 16)
        st_pool = pool("st", 8)
        yt_pool = pool("yt", 8)
        o_pool = pool("o", 3)
        ps_a = ctx.enter_context(tc.tile_pool(name="psa", bufs=4, space="PSUM"))
        ps_b = ctx.enter_context(tc.tile_pool(name="psb", bufs=4, space="PSUM"))

        # --- PE clock warmup: the tensor engine starts at a low p-state and
        # only reaches full clock after ~3us of continuous work. Burn that
        # ramp on dummy matmuls while the first input DMAs are in flight, so
        # the real prologue runs at full speed. ---
        warm = const_pool.tile([128, 512], F16, tag="warm", name="warm")
        nc.vector.memset(warm[:], 0.0)
        wpsum = ps_a.tile([128, 512], F32, tag="a", name="wpsum")
        for _ in range(11):
            nc.tensor.matmul(
                wpsum[:], warm[:, 0:128], warm[:], start=True, stop=True
            )

        # --- input DMAs first: x chunks on Scalar, weights on Sync (both
        # hardware DGE queues), in PE consumption order. Weights go on Sync
        # because DGE-ring backpressure parks the issuing engine for ~20us;
        # Sync has no early compute, while Scalar must run the Q-pass psum
        # evictions and EXPs from ~13us on. ---
        xt_all = [None] * NCHUNK

        def prefetch_xt(r):
            xtt = xt_pool.tile([128, C], F16, tag="xt", name="xtt")
            nc.scalar.dma_start(xtt[:], xs.ap()[r * 128 : (r + 1) * 128, :])
            xt_all[r] = xtt

        for r in range(5):
            prefetch_xt(r)

        def load_w2(pool_, dram, j, nm):
            # [256, 1024] DRAM rows -> one [128, 2048] tile (2 cb blocks)
            t = pool_.tile([128, 2 * C], F16, tag=nm, name=f"{nm}{j}")
            nc.sync.dma_start(
                t[:].rearrange("p (two f) -> p two f", two=2),
                dram.ap()[j * 256 : (j + 1) * 256, :].rearrange(
                    "(two p) f -> p two f", two=2
                ),
            )
            return t

        def load_w4(pool_, dram, g, nm):
            # [512, 1024] DRAM rows -> one [128, 4096] tile (4 cb blocks)
            t = pool_.tile([128, 4 * C], F16, tag=nm, name=f"{nm}{g}")
            nc.sync.dma_start(
                t[:].rearrange("p (four f) -> p four f", four=4),
                dram.ap()[g * 512 : (g + 1) * 512, :].rearrange(
                    "(four p) f -> p four f", four=4
                ),
            )
            return t

        def load_w1(pool_, dram, cb, nm):
            # single-cb granule: smallest arrival quantum for the first blocks
            t = pool_.tile([128, C], F16, tag=nm, name=f"{nm}{cb}")
            nc.sync.dma_start(t[:], dram.ap()[cb * 128 : (cb + 1) * 128, :])
            return t

        wq_sb = [load_w1(wq_pool, wq, cb, "wq") for cb in range(8)]
        wk_sb = [load_w2(wk_pool, wk, j, "wk") for j in range(4)]
        wv_sb = [load_w2(wv_pool, wv, j, "wv") for j in range(4)]
        wo_sb = [load_w4(wo_pool, wo, g, "wo") for g in range(2)]

        def wq_blk(cb, i):   # Q-proj rhs: cols i*512..+512 of cb's Q block
            return wq_sb[cb][:, i * 512 : (i + 1) * 512]

        def wk_blk(cb, i):
            return wk_sb[cb // 2][:, (cb % 2) * C + i * 512 : (cb % 2) * C + (i + 1) * 512]

        def wv_blk(cb, hp):  # V-proj lhsT: head-pair hp cols of cb's block
            base = (cb % 2) * C + hp * 128
            return wv_sb[cb // 2][:, base : base + 128]

        def wo_blk(cb, mi):
            base = (cb % 4) * C + mi * 512
            return wo_sb[cb // 4][:, base : base + 512]

        # --- constants (vector/gpsimd streams; after DMA issues) ---
        idf16 = const_pool.tile([128, 128], F16, tag="idf16", name="idf16")
        masks.make_identity(nc, idf16[:])
        ones = const_pool.tile([1, 128], F16, tag="ones", name="ones")
        nc.vector.memset(ones[:], 1.0)
        bq_sb = const_pool.tile([1, 3 * C], F16, tag="bq", name="bq_sb")
        bo_sb = const_pool.tile([1, C], F16, tag="bo", name="bo_sb")
        if with_bias:
            nc.gpsimd.dma_start(bq_sb[:], bqkv.ap().rearrange("(a f) -> a f", a=1))
            nc.gpsimd.dma_start(bo_sb[:], bout.ap().rearrange("(a f) -> a f", a=1))

        def xt_blk(r, cb):
            return xt_all[r][:, cb * 128 : (cb + 1) * 128]

        q_sb = [None] * NCHUNK
        k_sb = [None] * NCHUNK
        vt_sb = [None] * NCHUNK   # [e-pair 128, hp*128 + tok]
        wstate = [
            {"pe4": [None, None], "rs4": [None, None], "pn": [None] * 8}
            for _ in range(NWIN)
        ]

        def q_units(r):
            """Chunk r Q projection: 2 psum banks, cb-progressive."""
            st = {}

            def u_alloc():
                st["p"] = [
                    ps_a.tile([128, 512], F32, tag="a", name=f"pq{i}")
                    for i in range(2)
                ]

            def u_cb(cb):
                def f():
                    for i in range(2):
                        nc.tensor.matmul(
                            st["p"][i][:],
                            xt_blk(r, cb),
                            wq_blk(cb, i),
                            start=(cb == 0),
                            stop=(not with_bias and cb == 7),
                        )
                return f

            def u_fin():
                if with_bias:
                    for i in range(2):
                        nc.tensor.matmul(
                            st["p"][i][:],
                            ones[:, :],
                            bq_sb[:, i * 512 : (i + 1) * 512],
                            start=False,
                            stop=True,
                        )
                qt = q_pool.tile([128, C], F16, tag="q", name="qt")
                nc.scalar.mul(qt[:, 0:512], st["p"][0][:], 0.125)
                nc.scalar.mul(qt[:, 512:1024], st["p"][1][:], 0.125)
                q_sb[r] = qt

            return [u_alloc] + [u_cb(cb) for cb in range(8)] + [u_fin]

        def k_units(r):
            st = {}

            def u_alloc():
                st["p"] = [
                    ps_a.tile([128, 512], F32, tag="a", name=f"pk{i}")
                    for i in range(2)
                ]

            def u_cb(cb):
                def f():
                    for i in range(2):
                        nc.tensor.matmul(
                            st["p"][i][:],
                            xt_blk(r, cb),
                            wk_blk(cb, i),
                            start=(cb == 0),
                            stop=(not with_bias and cb == 7),
                        )
                return f

            def u_fin():
                if with_bias:
                    for i in range(2):
                        nc.tensor.matmul(
                            st["p"][i][:],
                            ones[:, :],
                            bq_sb[:, C + i * 512 : C + (i + 1) * 512],
                            start=False,
                            stop=True,
                        )
                kt = k_pool.tile([128, C], F16, tag="k", name="kt")
                nc.vector.tensor_copy(kt[:, 0:512], st["p"][0][:])
                nc.vector.tensor_copy(kt[:, 512:1024], st["p"][1][:])
                k_sb[r] = kt

            return [u_alloc] + [u_cb(cb) for cb in range(8)] + [u_fin]

        def v_units(r, ps=None, tag="b"):
            """Chunk r V^T projection. hp-outer: each psum slot's
            accumulation group (cb 0..7) runs without another group
            interleaving on the same bank — PSUM accumulation groups must
            be sequential per bank."""
            st = {}
            ps_pool = ps if ps is not None else ps_b

            def u_alloc():
                st["p"] = [
                    ps_pool.tile([128, 512], F32, tag=tag, name=f"pv{i}")
                    for i in range(2)
                ]

            def u_hp(hp):
                def f():
                    pv = st["p"][hp // 4]
                    sl = (hp % 4) * 128
                    for cb in range(8):
                        nc.tensor.matmul(
                            pv[:, sl : sl + 128],
                            wv_blk(cb, hp),
                            xt_blk(r, cb),
                            start=(cb == 0),
                            stop=(not with_bias and cb == 7),
                        )
                    if with_bias:
                        nc.tensor.matmul(
                            pv[:, sl : sl + 128],
                            bq_sb[:, 2 * C + hp * 128 : 2 * C + (hp + 1) * 128],
                            ones[:, :],
                            start=False,
                            stop=True,
                        )
                return f

            def u_fin():
                v_t = vt_pool.tile([128, C], F16, tag="vt", name="v_t")
                nc.vector.tensor_copy(v_t[:, 0:512], st["p"][0][:])
                nc.scalar.copy(v_t[:, 512:1024], st["p"][1][:])
                vt_sb[r] = v_t

            return [u_alloc] + [u_hp(hp) for hp in range(8)] + [u_fin]

        def qkv_units(r):
            units = []
            if r + 2 < NCHUNK and r + 2 >= 5:
                units.append(lambda: prefetch_xt(r + 2))
            units += q_units(r) + k_units(r) + v_units(r)
            return units

        def sphase_units(w):
            """Window w scores: S matmuls (4 head-pairs per PSUM bank),
            batched EXP / rowsum / reciprocal. Results land in SBUF for
            next round's phase2."""
            ws = wstate[w]

            def u_sb(j):
                def f():
                    sbt = ps_b.tile([128, 512], F32, tag="b", name="sbt")
                    for ii in range(4):
                        hp = 4 * j + ii
                        s = sbt[:, ii * 128 : (ii + 1) * 128]
                        for rr, (b0, b1) in (
                            (w, (True, False)),
                            (w + 1, (False, True)),
                        ):
                            nc.tensor.matmul(
                                s,
                                q_sb[rr][:, hp * 128 : (hp + 1) * 128],
                                k_sb[rr][:, hp * 128 : (hp + 1) * 128],
                                start=b0,
                                stop=b1,
                            )
                    pe4 = at_pool.tile([128, 256], F16, tag="pe4", name="pe4")
                    sb3 = sbt[:].rearrange("p (h c) -> p h c", h=4)
                    pe3 = pe4[:].rearrange("p (h e) -> p h e", h=4)
                    nc.scalar.activation(pe3[0:64], sb3[0:64, :, 0:64], EXP)
                    nc.scalar.activation(pe3[64:128], sb3[64:128, :, 64:128], EXP)
                    ssum4 = st_pool.tile([128, 4], F32, tag="ssum", name="ssum4")
                    nc.vector.reduce_sum(ssum4[:], pe3, axis=AXX)
                    rs4 = st_pool.tile([128, 4], F32, tag="rs", name="rs4")
                    nc.vector.reciprocal(rs4[:], ssum4[:])
                    ws["pe4"][j] = pe4
                    ws["rs4"][j] = rs4
                    # normalize now (DVE), a full round before the P^T
                    # matmuls read it — they never wait on a fresh DVE op
                    for ii in range(4):
                        p_n = pn_pool.tile([128, 64], F16, tag="p_n", name="p_n")
                        nc.vector.tensor_scalar_mul(
                            p_n[:],
                            pe4[:, ii * 64 : (ii + 1) * 64],
                            rs4[:, ii : ii + 1],
                        )
                        ws["pn"][4 * j + ii] = p_n
                return f

            return [u_sb(0), u_sb(1)]

        def phase2_units(w):
            """Window w: normalize, P^T, O, out-projection (chain results
            from last round's S-phase)."""
            ws = wstate[w]
            yt2 = [None] * 4
            ptsb = [None] * 4
            ptw = [None] * 2
            yw = [None] * 4

            def u_pt(pp):
                def f():
                    if pp % 2 == 0:
                        ptw[pp // 2] = ps_b.tile(
                            [128, 512], F32, tag="b", name="ptw"
                        )
                    ptp2 = ptw[pp // 2][:, (pp % 2) * 128 : (pp % 2) * 128 + 128]
                    for i in (0, 1):
                        p_n = ws["pn"][2 * pp + i]
                        nc.tensor.matmul(
                            ptp2[0:64, i * 64 : (i + 1) * 64],
                            p_n[0:64, :],
                            idf16[0:64, 0:64],
                            start=True,
                            stop=True,
                            tile_position=(0, 0),
                        )
                        nc.tensor.matmul(
                            ptp2[64:128, i * 64 : (i + 1) * 64],
                            p_n[64:128, :],
                            idf16[64:128, 64:128],
                            start=True,
                            stop=True,
                            tile_position=(64, 64),
                        )
                    pt2 = at_pool.tile([128, 128], F16, tag="ptsb", name="pt2")
                    eng = nc.vector.tensor_copy if pp % 2 else nc.scalar.copy
                    eng(pt2[:], ptp2)
                    ptsb[pp] = pt2
                return f

            def u_o(hp):
                def f():
                    if hp % 2 == 0:
                        yw[hp // 2] = ps_b.tile(
                            [128, 512], F32, tag="b", name="yw"
                        )
                    ypsum = yw[hp // 2][:, (hp % 2) * 256 : (hp % 2) * 256 + 256]
                    pt2 = ptsb[hp // 2]
                    c0 = (hp % 2) * 64
                    for po in (0, 64):
                        rh = pt2[po : po + 64, c0 : c0 + 64]
                        for wq_ in range(4):
                            vtt = vt_sb[w + wq_ // 2]
                            col = hp * 128 + (wq_ % 2) * 64
                            nc.tensor.matmul(
                                ypsum[po : po + 64, wq_ * 64 : (wq_ + 1) * 64],
                                vtt[po : po + 64, col : col + 64],
                                rh,
                                start=True,
                                stop=True,
                                tile_position=(po, po),
                            )
                    if hp % 2 == 1:
                        # Y^T[c, g*256 + d*4+wq] = yw[c, g*256 + wq*64+d]
                        ytt = yt_pool.tile([128, 512], F16, tag="yt", name="ytt")
                        eng = (
                            nc.vector.tensor_copy
                            if (hp // 2) % 2
                            else nc.scalar.copy
                        )
                        eng(
                            ytt[:].rearrange("p (g b a) -> p g a b", g=2, a=4),
                            yw[hp // 2][:].rearrange(
                                "p (g a b) -> p g a b", g=2, a=4
                            ),
                        )
                        yt2[hp // 2] = ytt
                return f

            def u_op(th):
                def f():
                    po_m = [
                        ps_a.tile([128, 512], F32, tag="a", name=f"pom{i}")
                        for i in range(2)
                    ]
                    for cb in range(8):
                        lh = yt2[cb // 2][
                            :, (cb % 2) * 256 + th * 128 : (cb % 2) * 256 + th * 128 + 128
                        ]
                        for mi in range(2):
                            nc.tensor.matmul(
                                po_m[mi][:],
                                lh,
                                wo_blk(cb, mi),
                                start=(cb == 0),
                                stop=(not with_bias and cb == 7),
                            )
                    if with_bias:
                        for mi in range(2):
                            nc.tensor.matmul(
                                po_m[mi][:],
                                ones[:, :],
                                bo_sb[:, mi * 512 : (mi + 1) * 512],
                                start=False,
                                stop=True,
                            )
                    ot = o_pool.tile([128, C], F32, tag="o", name="ot")
                    nc.vector.tensor_copy(ot[:, 0:512], po_m[0][:])
                    nc.scalar.copy(ot[:, 512:1024], po_m[1][:])
                    row = w * 256 + th * 128
                    nc.sync.dma_start(out.ap()[row : row + 128, :], ot[:])
                return f

            return [
                u_pt(0), u_pt(1), u_o(0), u_o(1), u_o(2), u_o(3),
                u_pt(2), u_pt(3), u_o(4), u_o(5), u_o(6), u_o(7),
                u_op(0), u_op(1),
            ]

        # --- prologue: chunks 0-2, interleaved pairwise per projection pass
        # so the PE consumes weight blocks as they arrive (wq -> wk -> wv
        # arrival order); V2 is deferred to round 3 (first use: round 4) ---
        def emit_pair(u0, u1):
            units = [u0[0], u1[0]]                   # allocs (2+2 psum banks)
            for i in range(8):
                units += [u0[1 + i], u1[1 + i]]
            units += [u0[9], u1[9]]                  # fins (evict -> banks free)
            return units

        # All Q passes before any K pass: Q only needs wq (first in the DMA
        # queue), so the PE never sits behind a K matmul waiting for the wk
        # tail. sphase(0)/(1) run before the V allocs (their sbt psum banks
        # are claimed by V0/V1, so the EXP readers must be emitted first).
        for u in (
            emit_pair(q_units(0), q_units(1))
            + emit_pair(q_units(2), q_units(3))
            + emit_pair(k_units(0), k_units(1))
            + sphase_units(0)
            + emit_pair(k_units(2), k_units(3))
            + sphase_units(1)
            + emit_pair(v_units(0), v_units(1))
        ):
            u()

        # --- steady state. Round 3 projects only V2/V3 (Q/K of chunks 2-3
        # ran in the prologue; V3 uses ps_a, which round 3's Q/K no longer
        # occupy, keeping psum reuse gated by emitted evictions). Rounds
        # 4..8 run full QKV. sphase(7) rides round 8's tail; the final
        # round interleaves phase2(6)/(7) with a head start for (6) ---
        for u in v_units(2):
            u()
        r3 = [lambda: prefetch_xt(5)] + v_units(3, ps=ps_a, tag="a")
        for u in interleave_blk(r3, phase2_units(0), 8, 4):
            u()
        ph6 = phase2_units(NWIN - 2)
        for r in range(4, NCHUNK):
            win = sphase_units(r - 2) + phase2_units(r - 3)
            if r == NCHUNK - 1:
                # window 7's S-phase and the first half of window 6's
                # phase2 ride round 8's tail, shrinking the final round
                win += sphase_units(NWIN - 1) + ph6[:6]
            for u in interleave_blk(qkv_units(r), win, 8, 4):
                u()
        ph7 = phase2_units(NWIN - 1)
        for u in interleave(ph6[6:], ph7):
            u()

    nc.compile()
    return nc


_CACHE = {}
_LOCK = threading.Lock()


def _get_program(with_bias=False):
    key = f"nc_bias{with_bias}"
    with _LOCK:
        if key not in _CACHE:
            _CACHE[key] = build_program(with_bias=with_bias)
        return _CACHE[key]


def make_in_maps(x, W_qkv, b_qkv, W_out, b_out):
    x16 = np.asarray(x, dtype=np.float16)
    wqkv16 = np.asarray(W_qkv, dtype=np.float16)
    wout16 = np.ascontiguousarray(np.asarray(W_out, dtype=np.float16))
    bqkv = np.asarray(b_qkv, dtype=np.float32)
    bout = np.asarray(b_out, dtype=np.float32)
    wq16 = np.ascontiguousarray(wqkv16[:, 0:C])
    wk16 = np.ascontiguousarray(wqkv16[:, C : 2 * C])
    wv16 = np.ascontiguousarray(wqkv16[:, 2 * C : 3 * C])
    in_maps = []
    for cid in range(8):
        b, half = cid // 2, cid % 2
        t0 = half * NWIN * STEP
        # xs_prep[r*128+p, cb*128+t] = x[b, t0 + r*128 + t, cb*128 + p]
        xT = x16[b, t0 : t0 + TOK, :].T                      # [C, TOK]
        xs_prep = np.ascontiguousarray(
            xT.reshape(8, 128, NCHUNK, 128).transpose(2, 1, 0, 3).reshape(TOK, C)
        )
        in_maps.append(
            {
                "xs": xs_prep,
                "wq": wq16,
                "wk": wk16,
                "wv": wv16,
                "wo": wout16,
                "bqkv": bqkv,
                "bout": bout,
            }
        )
    return in_maps


def kernel(x, W_qkv, b_qkv, W_out, b_out):
    with_bias = bool(np.any(b_qkv)) or bool(np.any(b_out))
    nc = _get_program(with_bias=with_bias)
    in_maps = make_in_maps(x, W_qkv, b_qkv, W_out, b_out)
    res = run_bass_kernel_spmd(nc, in_maps, core_ids=list(range(8)))
    out_full = np.empty((B, L, C), dtype=np.float32)
    for cid in range(8):
        b, half = cid // 2, cid % 2
        out_full[b, half * OUT_ROWS : (half + 1) * OUT_ROWS, :] = res.results[cid][
            "out"
        ]
    return out_full
